# revision 26
# baseline (speedup 1.0000x reference)
import numpy as np
import ml_dtypes
from contextlib import ExitStack

import concourse.mybir as mybir
import concourse.bass as bass
import concourse.tile as tile
from concourse.bass_utils import run_bass_kernel_spmd

# nn_Predictor (moe_routing). L=6 streams, B=16384, D=512, NC=3992, 4 experts.
# Strategy: gate on host (fp64), then route: each token runs ONLY its selected
# expert. Host sorts tokens by expert and deals them round-robin across the 8
# cores so every core gets identical per-expert segment sizes (SPMD program).
# Host pre-transposes activations to [feat, tok] and folds W2_e @ dec_W1 into
# one 512x512 matrix per expert, so the device runs three matmul stages:
# W1+relu, Me+tanh, dec2. All matmuls fp8 e4m3 in DoubleRow mode (K=256 per
# instruction, 2x bf16 throughput) with fp32 PSUM accumulation:
#  - weights are pre-scaled x32 on host so their ~0.02-rms entries land in
#    e4m3's normal range (min normal 2^-6; unscaled they quantize on the
#    2^-9 denormal grid). The 1/32 is folded into the activation `scale`
#    input (stages 1-2) and the host-side post-scale (stage 3) - exact.
#  - sigmoid(x) = 0.5 + 0.5*tanh(x/2): the device computes the centered
#    tanh (|values| ~0.06 instead of ~0.5), and the 0.5*colsum(dec_W2)
#    rank-1 term is added on host. This keeps stage-3 fp8 quantization and
#    activation-table error ~10x smaller than quantizing raw sigmoid.
# Schedule: all input DMAs are issued up-front (few, large, priority-ordered)
# and dec2 row-tiles are interleaved after each expert segment so the PSUM
# drain (vector+scalar copies) and the output DMA spread across the whole
# kernel instead of piling up in a back-loaded dec2 phase.
L, B, D, NCLS, NE = 6, 16384, 512, 3992, 4
NCORES = 8
KD = L * D                    # 3072 flat features
NKE = [12, 12, 24, 24]        # 128-row K chunks per expert
KLO = [0, 12, 0, 0]           # first K chunk per expert (front/back/all/all)
NCH = (NCLS + 511) // 512     # 8 output column chunks (last = 408)
EORDER = [0, 2, 3, 1]
WS = 32.0                     # weight pre-scale (host)

F32 = mybir.dt.float32
BF16 = mybir.dt.bfloat16
FP8 = mybir.dt.float8e4
BF = ml_dtypes.bfloat16
E4 = ml_dtypes.float8_e4m3
DR = mybir.MatmulPerfMode.DoubleRow


def _build(segs):
    # segs in EORDER processing order; TP need not be 128-aligned
    TP = sum(segs)
    nc = bass.Bass("TRN2")

    # xT: concatenation over EORDER segments of [128, nk_e, seg] blocks
    # (partition-major) so every activation load is a single linear DMA.
    xtot = 128 * sum(NKE[e] * sg for e, sg in zip(EORDER, segs))
    xT = nc.dram_tensor("xT", [xtot], FP8, kind="ExternalInput")
    w1_in = [
        nc.dram_tensor(f"w1_{e}", [128, NKE[e], 512], FP8, kind="ExternalInput")
        for e in range(NE)
    ]
    me_in = nc.dram_tensor("me", [128, NE * 4, 512], FP8, kind="ExternalInput")
    dw2_in = nc.dram_tensor("dw2", [128, 4, NCLS], FP8, kind="ExternalInput")
    bc_in = nc.dram_tensor("bc", [128, NE * 8], F32, kind="ExternalInput")
    out = nc.dram_tensor("out", [TP, NCLS], BF16, kind="ExternalOutput")

    with tile.TileContext(nc) as tc, ExitStack() as ctx:
        singles = ctx.enter_context(tc.tile_pool(name="singles", bufs=1))
        hP = ctx.enter_context(tc.tile_pool(name="hP", bufs=3))
        otP = ctx.enter_context(tc.tile_pool(name="otP", bufs=4))

        w1Ps = ctx.enter_context(tc.tile_pool(name="w1Ps", bufs=2, space="PSUM"))
        zPs = ctx.enter_context(tc.tile_pool(name="zPs", bufs=2, space="PSUM"))
        dPs = ctx.enter_context(tc.tile_pool(name="dPs", bufs=4, space="PSUM"))

        # All weights + activations fit in SBUF simultaneously (~135 KB of
        # 208), so everything is a single-buffered tile and every DMA is
        # issued up-front, ordered by first use. Each dma_start is a ~0.6us
        # serial trigger on the sync engine; one HWDGE DMA already fans
        # across all 16 SDMA engines, so use few, large transfers. Only the
        # first expert's chunks are split so the PE starts sooner.
        e0 = EORDER[0]
        w1sb = {e: singles.tile([128, NKE[e], 512], FP8, name=f"w1sb{e}")
                for e in range(NE)}
        xts = [singles.tile([128, NKE[e], sg], FP8, name=f"xt{ei}")
               for ei, (e, sg) in enumerate(zip(EORDER, segs)) if sg > 0]
        sig = singles.tile([128, 4, TP], FP8, name="sig")
        bcsb = singles.tile([128, NE * 8], F32)
        mesb = singles.tile([128, NE * 4, 512], FP8)
        dw2sb = singles.tile([128, 4, NCLS], FP8)

        xoffs = []
        xoff = 0
        for e, sg in zip(EORDER, segs):
            xoffs.append(xoff)
            xoff += 128 * NKE[e] * sg

        # priority order on the sync queue: expert-0 first pieces, bias and
        # expert-0's Me slice (stage-2 needs them by ~6us), the rest of
        # expert 0, then the remaining experts in processing order with
        # dec2 weights after the second expert's.
        # TRN2 has two HWDGE queues (SP + Activation), each processing
        # trigger (~0.6us) + transfer serially. Startup-critical pieces
        # (expert 0 + its Me slice) go first on the sync queue - measured:
        # the scalar queue's first transfers complete late, so it only
        # carries bulk loads needed later (the other experts' activations,
        # the rest of Me, dec2 weights), halving the mid-kernel stream.
        nk0, sg0 = NKE[e0], segs[0]
        for g in range(0, nk0, 6):
            nc.sync.dma_start(
                out=w1sb[e0][:, g : g + 6, :], in_=w1_in[e0][:, g : g + 6, :]
            )
            nc.sync.dma_start(
                out=xts[0][:, g : g + 6, :],
                in_=bass.AP(tensor=xT, offset=xoffs[0] + g * sg0,
                            ap=[[nk0 * sg0, 128], [1, 6 * sg0]]),
            )
        nc.sync.dma_start(out=bcsb, in_=bc_in[:, :])
        nc.sync.dma_start(
            out=mesb[:, e0 * 4 : (e0 + 1) * 4, :],
            in_=me_in[:, e0 * 4 : (e0 + 1) * 4, :],
        )
        assert e0 == 0
        nc.sync.dma_start(
            out=mesb[:, 4 : NE * 4, :], in_=me_in[:, 4 : NE * 4, :]
        )
        for ei, e in enumerate(EORDER):
            if ei == 0 or segs[ei] == 0:
                continue
            nk, sg = NKE[e], segs[ei]
            nc.sync.dma_start(out=w1sb[e][:, :, :], in_=w1_in[e][:, :, :])
            nc.sync.dma_start(
                out=xts[ei][:, :, :],
                in_=bass.AP(tensor=xT, offset=xoffs[ei],
                            ap=[[nk * sg, 128], [1, nk * sg]]),
            )
            if ei == 1:
                nc.sync.dma_start(out=dw2sb, in_=dw2_in[:, :, :])

        def dec2_tile(s):
            # one 128-token row tile of the decoder output; psums in pairs of
            # column chunks (two banks) so one copy drains 1024 columns,
            # alternating vector/scalar
            ot = otP.tile([128, NCLS], BF16, name="ot")
            for n in range(NCH):
                nw = min(512, NCLS - n * 512)
                ps = dPs.tile([128, 512], F32, name="dps")
                for i in range(2):
                    nc.tensor.matmul(
                        ps[:, :nw],
                        sig[:, 2 * i : 2 * i + 2, s : s + 128],
                        dw2sb[:, 2 * i : 2 * i + 2, n * 512 : n * 512 + nw],
                        start=(i == 0),
                        stop=(i == 1),
                        perf_mode=DR,
                    )
                if n % 8 < 5:
                    nc.vector.tensor_copy(
                        out=ot[:, n * 512 : n * 512 + nw], in_=ps[:, :nw]
                    )
                else:
                    nc.scalar.activation(
                        ot[:, n * 512 : n * 512 + nw], ps[:, :nw],
                        mybir.ActivationFunctionType.Copy,
                    )
                if s == TP - 128 and n == 3:
                    # final row tile: ship the first half while the second
                    # half is still being copied, shortening the tail drain
                    nc.sync.dma_start(
                        out=out[s : s + 128, :2048], in_=ot[:, :2048]
                    )
            if s == TP - 128:
                nc.sync.dma_start(
                    out=out[s : s + 128, 2048:], in_=ot[:, 2048:]
                )
            else:
                nc.sync.dma_start(out=out[s : s + 128, :], in_=ot[:, :])

        pos = 0
        next_s = 0
        for ei, e in enumerate(EORDER):
            if segs[ei] == 0:
                continue
            nk = NKE[e]
            sg = segs[ei]
            xt = xts[ei]
            for t0 in range(0, sg, 512):
                tcw = min(512, sg - t0)
                h = hP.tile([128, 4, 512], FP8, name="h")
                for m in range(4):
                    ps = w1Ps.tile([128, 512], F32, name="w1ps")
                    for kj in range(0, nk, 2):
                        nc.tensor.matmul(
                            ps[:, :tcw],
                            w1sb[e][:, kj : kj + 2, m * 128 : (m + 1) * 128],
                            xt[:, kj : kj + 2, t0 : t0 + tcw],
                            start=(kj == 0),
                            stop=(kj == nk - 2),
                            perf_mode=DR,
                        )
                    nc.scalar.activation(
                        h[:, m, :tcw], ps[:, :tcw],
                        mybir.ActivationFunctionType.Relu,
                        bias=bcsb[:, e * 4 + m : e * 4 + m + 1], scale=1.0 / WS,
                    )
                for m2 in range(4):
                    zs = zPs.tile([128, 512], F32, name="zps")
                    for i in range(2):
                        nc.tensor.matmul(
                            zs[:, :tcw],
                            mesb[:, e * 4 + 2 * i : e * 4 + 2 * i + 2,
                                 m2 * 128 : (m2 + 1) * 128],
                            h[:, 2 * i : 2 * i + 2, :tcw],
                            start=(i == 0),
                            stop=(i == 1),
                            perf_mode=DR,
                        )
                    nc.scalar.activation(
                        sig[:, m2, pos + t0 : pos + t0 + tcw], zs[:, :tcw],
                        mybir.ActivationFunctionType.Tanh,
                        bias=bcsb[:, 16 + e * 4 + m2 : 16 + e * 4 + m2 + 1],
                        scale=0.5 / WS,
                    )
            pos += segs[ei]
            # emit dec2 for every fully-tanh'd 128-token row tile so copies
            # and output DMA overlap the next expert's matmuls
            while next_s + 128 <= pos:
                dec2_tile(next_s)
                next_s += 128

    import bass_rust

    bass_rust.generate_event_semaphores(nc)
    return nc


_NC_CACHE = {}


def _get_nc(segs):
    key = tuple(segs)
    if key not in _NC_CACHE:
        _NC_CACHE[key] = _build(key)
    return _NC_CACHE[key]


def _route(inputs):
    """Host gate + routing plan. Returns (segs in EORDER, idx, flat)."""
    f32 = np.float32
    x = np.asarray(inputs["fusion_hs"], f32)                  # [L, B, D]
    flat = np.transpose(x, (1, 0, 2)).reshape(B, KD)          # [B, 6D]
    logits = flat.astype(np.float64) @ np.asarray(
        inputs["gate_W"], f32
    ).astype(np.float64) + np.asarray(inputs["gate_b"], f32).astype(np.float64)
    am = np.argmax(logits, axis=1)
    idx = [np.where(am == e)[0] for e in range(NE)]
    segs = [(len(idx[e]) + NCORES - 1) // NCORES for e in EORDER]
    # shave device capacity to exactly B/NCORES tokens per core: a ragged
    # dec2 subtile costs full matmul time for a couple of tokens, so the
    # few overflow tokens are computed on host instead
    excess = sum(segs) - B // NCORES
    if excess > 0:
        segs[int(np.argmax(segs))] -= excess
    return segs, idx, flat, am


def _prep_inputs(inputs, segs, idx, flat):
    f32 = np.float32
    ws = f32(WS)

    w1_raw = [
        np.asarray(inputs["e0_W1"], f32),
        np.asarray(inputs["e1_W1"], f32),
        np.asarray(inputs["e2_W1"], f32),
        np.array(inputs["e3_W1"], f32, copy=True),
    ]
    w1_raw[3][: 3 * D] *= f32(np.asarray(inputs["e3_a"]).reshape(-1)[0])
    w1_raw[3][3 * D :] *= f32(np.asarray(inputs["e3_b"]).reshape(-1)[0])

    dw1 = np.asarray(inputs["dec_W1"], f32)
    db1 = np.asarray(inputs["dec_b1"], f32)

    common = {}
    bc_cols = np.empty((128, NE * 8), f32)
    me_host = np.empty((128, NE * 4, 512), E4)
    for e in range(NE):
        b1e = np.asarray(inputs[f"e{e}_b1"], f32)
        bc_cols[:, e * 4 : (e + 1) * 4] = b1e.reshape(4, 128).T
        mee = np.asarray(inputs[f"e{e}_W2"], f32) @ dw1        # [512hid, 512h2]
        cee = np.asarray(inputs[f"e{e}_b2"], f32) @ dw1 + db1  # [512]
        bc_cols[:, 16 + e * 4 : 16 + (e + 1) * 4] = 0.5 * cee.reshape(4, 128).T
        me_host[:, e * 4 : (e + 1) * 4, :] = (
            (ws * mee).reshape(4, 128, 512).transpose(1, 0, 2).astype(E4)
        )
        common[f"w1_{e}"] = np.ascontiguousarray(
            (ws * w1_raw[e]).astype(E4).reshape(NKE[e], 128, 512).transpose(1, 0, 2)
        )
    common["bc"] = bc_cols
    common["me"] = me_host
    common["dw2"] = np.ascontiguousarray(
        (ws * 0.5 * np.asarray(inputs["dec_W2"], f32))
        .astype(E4)
        .reshape(4, 128, NCLS)
        .transpose(1, 0, 2)
    )

    flatT = np.ascontiguousarray(flat.T.astype(E4))            # [3072, B]
    xtot = 128 * sum(NKE[e] * sg for e, sg in zip(EORDER, segs))
    in_maps = []
    overflow = []
    for c in range(NCORES):
        xbuf = np.empty(xtot, E4)
        xoff = 0
        for ei, e in enumerate(EORDER):
            ids = idx[e][c::NCORES]
            take = min(len(ids), segs[ei])
            sel = ids[:take]
            if segs[ei] > take:
                pad = np.full(segs[ei] - take, ids[0] if take else 0, np.int64)
                sel = np.concatenate([sel, pad])
            overflow.extend(ids[take:].tolist())
            nk = NKE[e]
            sg = segs[ei]
            blk = flatT[KLO[e] * 128 : (KLO[e] + nk) * 128, sel]   # [nk*128, sg]
            xbuf[xoff : xoff + 128 * nk * sg] = (
                blk.reshape(nk, 128, sg).transpose(1, 0, 2).ravel()
            )
            xoff += 128 * nk * sg
        m = dict(common)
        m["xT"] = xbuf
        in_maps.append(m)
    return in_maps, overflow


def _host_fwd(inputs, flat, am, toks):
    # exact fp32 reference math for the few overflow tokens
    f32 = np.float32
    res = np.empty((len(toks), NCLS), f32)
    a = f32(np.asarray(inputs["e3_a"]).reshape(-1)[0])
    b = f32(np.asarray(inputs["e3_b"]).reshape(-1)[0])
    dW1 = np.asarray(inputs["dec_W1"], f32)
    db1 = np.asarray(inputs["dec_b1"], f32)
    dW2 = np.asarray(inputs["dec_W2"], f32)
    db2 = np.asarray(inputs["dec_b2"], f32)
    for j, t in enumerate(toks):
        e = int(am[t])
        if e == 0:
            xin = flat[t, : 3 * D]
        elif e == 1:
            xin = flat[t, 3 * D :]
        elif e == 2:
            xin = flat[t]
        else:
            xin = np.concatenate([flat[t, : 3 * D] * a, flat[t, 3 * D :] * b])
        h = np.maximum(
            xin @ np.asarray(inputs[f"e{e}_W1"], f32)
            + np.asarray(inputs[f"e{e}_b1"], f32), 0
        )
        sel = h @ np.asarray(inputs[f"e{e}_W2"], f32) + np.asarray(
            inputs[f"e{e}_b2"], f32
        )
        z = 1.0 / (1.0 + np.exp(-(sel @ dW1 + db1)))
        res[j] = z @ dW2 + db2
    return res


def _finish(inputs, segs, idx, results, host_rows, overflow):
    f32 = np.float32
    dw2_f = np.asarray(inputs["dec_W2"], f32)
    bias_out = 0.5 * dw2_f.sum(axis=0) + np.asarray(inputs["dec_b2"], f32)
    out_full = np.empty((B, NCLS), f32)
    for c in range(NCORES):
        res = np.asarray(results[c]["out"]).astype(f32)        # [TP, NCLS]
        p = 0
        for ei, e in enumerate(EORDER):
            ids = idx[e][c::NCORES]
            take = min(len(ids), segs[ei])
            out_full[ids[:take]] = res[p : p + take]
            p += segs[ei]
    out_full *= f32(1.0 / WS)
    out_full += bias_out
    if overflow:
        out_full[overflow] = host_rows   # host rows already include the bias
    return out_full


def _run(inputs, trace=False, tmpdir=None):
    segs, idx, flat, am = _route(inputs)
    nc = _get_nc(segs)
    in_maps, overflow = _prep_inputs(inputs, segs, idx, flat)
    res = run_bass_kernel_spmd(
        nc, in_maps, core_ids=list(range(NCORES)), trace=trace, tmpdir=tmpdir
    )
    host_rows = _host_fwd(inputs, flat, am, overflow) if overflow else None
    out = _finish(inputs, segs, idx, res.results, host_rows, overflow)
    return out, res


def kernel(**inputs):
    out, _ = _run(inputs)
    return out


# revision 27
# speedup vs baseline: 1.0175x; 1.0175x over previous
import numpy as np
import ml_dtypes
from contextlib import ExitStack

import concourse.mybir as mybir
import concourse.bass as bass
import concourse.tile as tile
from concourse.bass_utils import run_bass_kernel_spmd

# nn_Predictor (moe_routing). L=6 streams, B=16384, D=512, NC=3992, 4 experts.
# Strategy: gate on host (fp64), then route: each token runs ONLY its selected
# expert. Host sorts tokens by expert and deals them round-robin across the 8
# cores so every core gets identical per-expert segment sizes (SPMD program).
# Host pre-transposes activations to [feat, tok] and folds W2_e @ dec_W1 into
# one 512x512 matrix per expert, so the device runs three matmul stages:
# W1+relu, Me+tanh, dec2. All matmuls fp8 e4m3 in DoubleRow mode (K=256 per
# instruction, 2x bf16 throughput) with fp32 PSUM accumulation:
#  - weights are pre-scaled x32 on host so their ~0.02-rms entries land in
#    e4m3's normal range (min normal 2^-6; unscaled they quantize on the
#    2^-9 denormal grid). The 1/32 is folded into the activation `scale`
#    input (stages 1-2) and the host-side post-scale (stage 3) - exact.
#  - sigmoid(x) = 0.5 + 0.5*tanh(x/2): the device computes the centered
#    tanh (|values| ~0.06 instead of ~0.5), and the 0.5*colsum(dec_W2)
#    rank-1 term is added on host. This keeps stage-3 fp8 quantization and
#    activation-table error ~10x smaller than quantizing raw sigmoid.
# Schedule: all input DMAs are issued up-front (few, large, priority-ordered)
# and dec2 row-tiles are interleaved after each expert segment so the PSUM
# drain (vector+scalar copies) and the output DMA spread across the whole
# kernel instead of piling up in a back-loaded dec2 phase.
L, B, D, NCLS, NE = 6, 16384, 512, 3992, 4
NCORES = 8
KD = L * D                    # 3072 flat features
NKE = [12, 12, 24, 24]        # 128-row K chunks per expert
KLO = [0, 12, 0, 0]           # first K chunk per expert (front/back/all/all)
NCH = (NCLS + 511) // 512     # 8 output column chunks (last = 408)
EORDER = [0, 2, 3, 1]
WS = 32.0                     # weight pre-scale (host)

F32 = mybir.dt.float32
BF16 = mybir.dt.bfloat16
FP8 = mybir.dt.float8e4
BF = ml_dtypes.bfloat16
E4 = ml_dtypes.float8_e4m3
DR = mybir.MatmulPerfMode.DoubleRow


def _build(segs):
    # segs in EORDER processing order; TP need not be 128-aligned
    TP = sum(segs)
    nc = bass.Bass("TRN2")

    # xT: concatenation over EORDER segments of [128, nk_e, seg] blocks
    # (partition-major) so every activation load is a single linear DMA.
    xtot = 128 * sum(NKE[e] * sg for e, sg in zip(EORDER, segs))
    xT = nc.dram_tensor("xT", [xtot], FP8, kind="ExternalInput")
    w1_in = [
        nc.dram_tensor(f"w1_{e}", [128, NKE[e], 512], FP8, kind="ExternalInput")
        for e in range(NE)
    ]
    me_in = nc.dram_tensor("me", [128, NE * 4, 512], FP8, kind="ExternalInput")
    dw2_in = nc.dram_tensor("dw2", [128, 4, NCLS], FP8, kind="ExternalInput")
    bc_in = nc.dram_tensor("bc", [128, NE * 8], F32, kind="ExternalInput")
    out = nc.dram_tensor("out", [TP, NCLS], BF16, kind="ExternalOutput")

    with tile.TileContext(nc) as tc, ExitStack() as ctx:
        singles = ctx.enter_context(tc.tile_pool(name="singles", bufs=1))
        hP = ctx.enter_context(tc.tile_pool(name="hP", bufs=3))
        otP = ctx.enter_context(tc.tile_pool(name="otP", bufs=4))

        w1Ps = ctx.enter_context(tc.tile_pool(name="w1Ps", bufs=2, space="PSUM"))
        zPs = ctx.enter_context(tc.tile_pool(name="zPs", bufs=2, space="PSUM"))
        dPs = ctx.enter_context(tc.tile_pool(name="dPs", bufs=4, space="PSUM"))

        # All weights + activations fit in SBUF simultaneously (~135 KB of
        # 208), so everything is a single-buffered tile and every DMA is
        # issued up-front, ordered by first use. Each dma_start is a ~0.6us
        # serial trigger on the sync engine; one HWDGE DMA already fans
        # across all 16 SDMA engines, so use few, large transfers. Only the
        # first expert's chunks are split so the PE starts sooner.
        e0 = EORDER[0]
        w1sb = {e: singles.tile([128, NKE[e], 512], FP8, name=f"w1sb{e}")
                for e in range(NE)}
        xts = [singles.tile([128, NKE[e], sg], FP8, name=f"xt{ei}")
               for ei, (e, sg) in enumerate(zip(EORDER, segs)) if sg > 0]
        sig = singles.tile([128, 4, TP], FP8, name="sig")
        bcsb = singles.tile([128, NE * 8], F32)
        mesb = singles.tile([128, NE * 4, 512], FP8)
        dw2sb = singles.tile([128, 4, NCLS], FP8)

        xoffs = []
        xoff = 0
        for e, sg in zip(EORDER, segs):
            xoffs.append(xoff)
            xoff += 128 * NKE[e] * sg

        # priority order on the sync queue: expert-0 first pieces, bias and
        # expert-0's Me slice (stage-2 needs them by ~6us), the rest of
        # expert 0, then the remaining experts in processing order with
        # dec2 weights after the second expert's.
        # TRN2 has two HWDGE queues (SP + Activation), each processing
        # trigger (~0.6us) + transfer serially. Startup-critical pieces
        # (expert 0 + its Me slice) go first on the sync queue - measured:
        # the scalar queue's first transfers complete late, so it only
        # carries bulk loads needed later (the other experts' activations,
        # the rest of Me, dec2 weights), halving the mid-kernel stream.
        nk0, sg0 = NKE[e0], segs[0]
        for g in range(0, nk0, 6):
            nc.sync.dma_start(
                out=w1sb[e0][:, g : g + 6, :], in_=w1_in[e0][:, g : g + 6, :]
            )
            nc.sync.dma_start(
                out=xts[0][:, g : g + 6, :],
                in_=bass.AP(tensor=xT, offset=xoffs[0] + g * sg0,
                            ap=[[nk0 * sg0, 128], [1, 6 * sg0]]),
            )
        nc.sync.dma_start(out=bcsb, in_=bc_in[:, :])
        nc.sync.dma_start(
            out=mesb[:, e0 * 4 : (e0 + 1) * 4, :],
            in_=me_in[:, e0 * 4 : (e0 + 1) * 4, :],
        )
        assert e0 == 0
        nc.sync.dma_start(
            out=mesb[:, 4 : NE * 4, :], in_=me_in[:, 4 : NE * 4, :]
        )
        for ei, e in enumerate(EORDER):
            if ei == 0 or segs[ei] == 0:
                continue
            nk, sg = NKE[e], segs[ei]
            nc.sync.dma_start(out=w1sb[e][:, :, :], in_=w1_in[e][:, :, :])
            nc.sync.dma_start(
                out=xts[ei][:, :, :],
                in_=bass.AP(tensor=xT, offset=xoffs[ei],
                            ap=[[nk * sg, 128], [1, nk * sg]]),
            )
            if ei == 1:
                nc.sync.dma_start(out=dw2sb, in_=dw2_in[:, :, :])

        def dec2_tile(s):
            # one 128-token row tile of the decoder output; psums in pairs of
            # column chunks (two banks) so one copy drains 1024 columns,
            # alternating vector/scalar
            ot = otP.tile([128, NCLS], BF16, name="ot")
            for n in range(NCH):
                nw = min(512, NCLS - n * 512)
                ps = dPs.tile([128, 512], F32, name="dps")
                for i in range(2):
                    nc.tensor.matmul(
                        ps[:, :nw],
                        sig[:, 2 * i : 2 * i + 2, s : s + 128],
                        dw2sb[:, 2 * i : 2 * i + 2, n * 512 : n * 512 + nw],
                        start=(i == 0),
                        stop=(i == 1),
                        perf_mode=DR,
                    )
                if n % 2 == 0:
                    nc.vector.tensor_copy(
                        out=ot[:, n * 512 : n * 512 + nw], in_=ps[:, :nw]
                    )
                else:
                    nc.scalar.activation(
                        ot[:, n * 512 : n * 512 + nw], ps[:, :nw],
                        mybir.ActivationFunctionType.Copy,
                    )
                if s == TP - 128 and n == 3:
                    # final row tile: ship the first half while the second
                    # half is still being copied, shortening the tail drain
                    nc.sync.dma_start(
                        out=out[s : s + 128, :2048], in_=ot[:, :2048]
                    )
            if s == TP - 128:
                nc.sync.dma_start(
                    out=out[s : s + 128, 2048:], in_=ot[:, 2048:]
                )
            else:
                nc.sync.dma_start(out=out[s : s + 128, :], in_=ot[:, :])

        pos = 0
        next_s = 0
        for ei, e in enumerate(EORDER):
            if segs[ei] == 0:
                continue
            nk = NKE[e]
            sg = segs[ei]
            xt = xts[ei]
            for t0 in range(0, sg, 512):
                tcw = min(512, sg - t0)
                h = hP.tile([128, 4, 512], FP8, name="h")
                for m in range(4):
                    ps = w1Ps.tile([128, 512], F32, name="w1ps")
                    for kj in range(0, nk, 2):
                        nc.tensor.matmul(
                            ps[:, :tcw],
                            w1sb[e][:, kj : kj + 2, m * 128 : (m + 1) * 128],
                            xt[:, kj : kj + 2, t0 : t0 + tcw],
                            start=(kj == 0),
                            stop=(kj == nk - 2),
                            perf_mode=DR,
                        )
                    nc.scalar.activation(
                        h[:, m, :tcw], ps[:, :tcw],
                        mybir.ActivationFunctionType.Relu,
                        bias=bcsb[:, e * 4 + m : e * 4 + m + 1], scale=1.0 / WS,
                    )
                for m2 in range(4):
                    zs = zPs.tile([128, 512], F32, name="zps")
                    for i in range(2):
                        nc.tensor.matmul(
                            zs[:, :tcw],
                            mesb[:, e * 4 + 2 * i : e * 4 + 2 * i + 2,
                                 m2 * 128 : (m2 + 1) * 128],
                            h[:, 2 * i : 2 * i + 2, :tcw],
                            start=(i == 0),
                            stop=(i == 1),
                            perf_mode=DR,
                        )
                    nc.scalar.activation(
                        sig[:, m2, pos + t0 : pos + t0 + tcw], zs[:, :tcw],
                        mybir.ActivationFunctionType.Tanh,
                        bias=bcsb[:, 16 + e * 4 + m2 : 16 + e * 4 + m2 + 1],
                        scale=0.5 / WS,
                    )
            pos += segs[ei]
            # emit dec2 for every fully-tanh'd 128-token row tile so copies
            # and output DMA overlap the next expert's matmuls
            while next_s + 128 <= pos:
                dec2_tile(next_s)
                next_s += 128

    import bass_rust

    bass_rust.generate_event_semaphores(nc)
    return nc


_NC_CACHE = {}


def _get_nc(segs):
    key = tuple(segs)
    if key not in _NC_CACHE:
        _NC_CACHE[key] = _build(key)
    return _NC_CACHE[key]


def _route(inputs):
    """Host gate + routing plan. Returns (segs in EORDER, idx, flat)."""
    f32 = np.float32
    x = np.asarray(inputs["fusion_hs"], f32)                  # [L, B, D]
    flat = np.transpose(x, (1, 0, 2)).reshape(B, KD)          # [B, 6D]
    logits = flat.astype(np.float64) @ np.asarray(
        inputs["gate_W"], f32
    ).astype(np.float64) + np.asarray(inputs["gate_b"], f32).astype(np.float64)
    am = np.argmax(logits, axis=1)
    idx = [np.where(am == e)[0] for e in range(NE)]
    segs = [(len(idx[e]) + NCORES - 1) // NCORES for e in EORDER]
    # shave device capacity to exactly B/NCORES tokens per core: a ragged
    # dec2 subtile costs full matmul time for a couple of tokens, so the
    # few overflow tokens are computed on host instead
    excess = sum(segs) - B // NCORES
    if excess > 0:
        segs[int(np.argmax(segs))] -= excess
    return segs, idx, flat, am


def _prep_inputs(inputs, segs, idx, flat):
    f32 = np.float32
    ws = f32(WS)

    w1_raw = [
        np.asarray(inputs["e0_W1"], f32),
        np.asarray(inputs["e1_W1"], f32),
        np.asarray(inputs["e2_W1"], f32),
        np.array(inputs["e3_W1"], f32, copy=True),
    ]
    w1_raw[3][: 3 * D] *= f32(np.asarray(inputs["e3_a"]).reshape(-1)[0])
    w1_raw[3][3 * D :] *= f32(np.asarray(inputs["e3_b"]).reshape(-1)[0])

    dw1 = np.asarray(inputs["dec_W1"], f32)
    db1 = np.asarray(inputs["dec_b1"], f32)

    common = {}
    bc_cols = np.empty((128, NE * 8), f32)
    me_host = np.empty((128, NE * 4, 512), E4)
    for e in range(NE):
        b1e = np.asarray(inputs[f"e{e}_b1"], f32)
        bc_cols[:, e * 4 : (e + 1) * 4] = b1e.reshape(4, 128).T
        mee = np.asarray(inputs[f"e{e}_W2"], f32) @ dw1        # [512hid, 512h2]
        cee = np.asarray(inputs[f"e{e}_b2"], f32) @ dw1 + db1  # [512]
        bc_cols[:, 16 + e * 4 : 16 + (e + 1) * 4] = 0.5 * cee.reshape(4, 128).T
        me_host[:, e * 4 : (e + 1) * 4, :] = (
            (ws * mee).reshape(4, 128, 512).transpose(1, 0, 2).astype(E4)
        )
        common[f"w1_{e}"] = np.ascontiguousarray(
            (ws * w1_raw[e]).astype(E4).reshape(NKE[e], 128, 512).transpose(1, 0, 2)
        )
    common["bc"] = bc_cols
    common["me"] = me_host
    common["dw2"] = np.ascontiguousarray(
        (ws * 0.5 * np.asarray(inputs["dec_W2"], f32))
        .astype(E4)
        .reshape(4, 128, NCLS)
        .transpose(1, 0, 2)
    )

    flatT = np.ascontiguousarray(flat.T.astype(E4))            # [3072, B]
    xtot = 128 * sum(NKE[e] * sg for e, sg in zip(EORDER, segs))
    in_maps = []
    overflow = []
    for c in range(NCORES):
        xbuf = np.empty(xtot, E4)
        xoff = 0
        for ei, e in enumerate(EORDER):
            ids = idx[e][c::NCORES]
            take = min(len(ids), segs[ei])
            sel = ids[:take]
            if segs[ei] > take:
                pad = np.full(segs[ei] - take, ids[0] if take else 0, np.int64)
                sel = np.concatenate([sel, pad])
            overflow.extend(ids[take:].tolist())
            nk = NKE[e]
            sg = segs[ei]
            blk = flatT[KLO[e] * 128 : (KLO[e] + nk) * 128, sel]   # [nk*128, sg]
            xbuf[xoff : xoff + 128 * nk * sg] = (
                blk.reshape(nk, 128, sg).transpose(1, 0, 2).ravel()
            )
            xoff += 128 * nk * sg
        m = dict(common)
        m["xT"] = xbuf
        in_maps.append(m)
    return in_maps, overflow


def _host_fwd(inputs, flat, am, toks):
    # exact fp32 reference math for the few overflow tokens
    f32 = np.float32
    res = np.empty((len(toks), NCLS), f32)
    a = f32(np.asarray(inputs["e3_a"]).reshape(-1)[0])
    b = f32(np.asarray(inputs["e3_b"]).reshape(-1)[0])
    dW1 = np.asarray(inputs["dec_W1"], f32)
    db1 = np.asarray(inputs["dec_b1"], f32)
    dW2 = np.asarray(inputs["dec_W2"], f32)
    db2 = np.asarray(inputs["dec_b2"], f32)
    for j, t in enumerate(toks):
        e = int(am[t])
        if e == 0:
            xin = flat[t, : 3 * D]
        elif e == 1:
            xin = flat[t, 3 * D :]
        elif e == 2:
            xin = flat[t]
        else:
            xin = np.concatenate([flat[t, : 3 * D] * a, flat[t, 3 * D :] * b])
        h = np.maximum(
            xin @ np.asarray(inputs[f"e{e}_W1"], f32)
            + np.asarray(inputs[f"e{e}_b1"], f32), 0
        )
        sel = h @ np.asarray(inputs[f"e{e}_W2"], f32) + np.asarray(
            inputs[f"e{e}_b2"], f32
        )
        z = 1.0 / (1.0 + np.exp(-(sel @ dW1 + db1)))
        res[j] = z @ dW2 + db2
    return res


def _finish(inputs, segs, idx, results, host_rows, overflow):
    f32 = np.float32
    dw2_f = np.asarray(inputs["dec_W2"], f32)
    bias_out = 0.5 * dw2_f.sum(axis=0) + np.asarray(inputs["dec_b2"], f32)
    out_full = np.empty((B, NCLS), f32)
    for c in range(NCORES):
        res = np.asarray(results[c]["out"]).astype(f32)        # [TP, NCLS]
        p = 0
        for ei, e in enumerate(EORDER):
            ids = idx[e][c::NCORES]
            take = min(len(ids), segs[ei])
            out_full[ids[:take]] = res[p : p + take]
            p += segs[ei]
    out_full *= f32(1.0 / WS)
    out_full += bias_out
    if overflow:
        out_full[overflow] = host_rows   # host rows already include the bias
    return out_full


def _run(inputs, trace=False, tmpdir=None):
    segs, idx, flat, am = _route(inputs)
    nc = _get_nc(segs)
    in_maps, overflow = _prep_inputs(inputs, segs, idx, flat)
    res = run_bass_kernel_spmd(
        nc, in_maps, core_ids=list(range(NCORES)), trace=trace, tmpdir=tmpdir
    )
    host_rows = _host_fwd(inputs, flat, am, overflow) if overflow else None
    out = _finish(inputs, segs, idx, res.results, host_rows, overflow)
    return out, res


def kernel(**inputs):
    out, _ = _run(inputs)
    return out


# revision 29
# speedup vs baseline: 1.0429x; 1.0249x over previous
import numpy as np
import ml_dtypes
from contextlib import ExitStack

import concourse.mybir as mybir
import concourse.bass as bass
import concourse.tile as tile
from concourse.bass_utils import run_bass_kernel_spmd

# nn_Predictor (moe_routing). L=6 streams, B=16384, D=512, NC=3992, 4 experts.
# Strategy: gate on host (fp64), then route: each token runs ONLY its selected
# expert. Host sorts tokens by expert and deals them round-robin across the 8
# cores so every core gets identical per-expert segment sizes (SPMD program).
# Host pre-transposes activations to [feat, tok] and folds W2_e @ dec_W1 into
# one 512x512 matrix per expert, so the device runs three matmul stages:
# W1+relu, Me+tanh, dec2. All matmuls fp8 e4m3 in DoubleRow mode (K=256 per
# instruction, 2x bf16 throughput) with fp32 PSUM accumulation:
#  - weights are pre-scaled x32 on host so their ~0.02-rms entries land in
#    e4m3's normal range (min normal 2^-6; unscaled they quantize on the
#    2^-9 denormal grid). The 1/32 is folded into the activation `scale`
#    input (stages 1-2) and the host-side post-scale (stage 3) - exact.
#  - sigmoid(x) = 0.5 + 0.5*tanh(x/2): the device computes the centered
#    tanh (|values| ~0.06 instead of ~0.5), and the 0.5*colsum(dec_W2)
#    rank-1 term is added on host. This keeps stage-3 fp8 quantization and
#    activation-table error ~10x smaller than quantizing raw sigmoid.
# Schedule: all input DMAs are issued up-front (few, large, priority-ordered)
# and dec2 row-tiles are interleaved after each expert segment so the PSUM
# drain (vector+scalar copies) and the output DMA spread across the whole
# kernel instead of piling up in a back-loaded dec2 phase.
L, B, D, NCLS, NE = 6, 16384, 512, 3992, 4
NCORES = 8
KD = L * D                    # 3072 flat features
NKE = [12, 12, 24, 24]        # 128-row K chunks per expert
KLO = [0, 12, 0, 0]           # first K chunk per expert (front/back/all/all)
NCH = (NCLS + 511) // 512     # 8 output column chunks (last = 408)
EORDER = [0, 2, 3, 1]
WS = 32.0                     # weight pre-scale (host)

F32 = mybir.dt.float32
BF16 = mybir.dt.bfloat16
FP8 = mybir.dt.float8e4
BF = ml_dtypes.bfloat16
E4 = ml_dtypes.float8_e4m3
DR = mybir.MatmulPerfMode.DoubleRow


def _build(segs):
    # segs in EORDER processing order; TP need not be 128-aligned
    TP = sum(segs)
    nc = bass.Bass("TRN2")

    # xT: concatenation over EORDER segments of [128, nk_e, seg] blocks
    # (partition-major) so every activation load is a single linear DMA.
    xtot = 128 * sum(NKE[e] * sg for e, sg in zip(EORDER, segs))
    xT = nc.dram_tensor("xT", [xtot], FP8, kind="ExternalInput")
    w1_in = [
        nc.dram_tensor(f"w1_{e}", [128, NKE[e], 512], FP8, kind="ExternalInput")
        for e in range(NE)
    ]
    me_in = nc.dram_tensor("me", [128, NE * 4, 512], FP8, kind="ExternalInput")
    dw2_in = nc.dram_tensor("dw2", [128, 4, NCLS], FP8, kind="ExternalInput")
    bc_in = nc.dram_tensor("bc", [128, NE * 8], F32, kind="ExternalInput")
    out = nc.dram_tensor("out", [TP, NCLS], BF16, kind="ExternalOutput")

    with tile.TileContext(nc) as tc, ExitStack() as ctx:
        singles = ctx.enter_context(tc.tile_pool(name="singles", bufs=1))
        hP = ctx.enter_context(tc.tile_pool(name="hP", bufs=3))
        otP = ctx.enter_context(tc.tile_pool(name="otP", bufs=4))

        w1Ps = ctx.enter_context(tc.tile_pool(name="w1Ps", bufs=2, space="PSUM"))
        zPs = ctx.enter_context(tc.tile_pool(name="zPs", bufs=2, space="PSUM"))
        dPs = ctx.enter_context(tc.tile_pool(name="dPs", bufs=4, space="PSUM"))

        # All weights + activations fit in SBUF simultaneously (~135 KB of
        # 208), so everything is a single-buffered tile and every DMA is
        # issued up-front, ordered by first use. Each dma_start is a ~0.6us
        # serial trigger on the sync engine; one HWDGE DMA already fans
        # across all 16 SDMA engines, so use few, large transfers. Only the
        # first expert's chunks are split so the PE starts sooner.
        e0 = EORDER[0]
        w1sb = {e: singles.tile([128, NKE[e], 512], FP8, name=f"w1sb{e}")
                for e in range(NE)}
        xts = [singles.tile([128, NKE[e], sg], FP8, name=f"xt{ei}")
               for ei, (e, sg) in enumerate(zip(EORDER, segs)) if sg > 0]
        sig = singles.tile([128, 4, TP], FP8, name="sig")
        bcsb = singles.tile([128, NE * 8], F32)
        mesb = singles.tile([128, NE * 4, 512], FP8)
        dw2sb = singles.tile([128, 4, NCLS], FP8)

        xoffs = []
        xoff = 0
        for e, sg in zip(EORDER, segs):
            xoffs.append(xoff)
            xoff += 128 * NKE[e] * sg

        # priority order on the sync queue: expert-0 first pieces, bias and
        # expert-0's Me slice (stage-2 needs them by ~6us), the rest of
        # expert 0, then the remaining experts in processing order with
        # dec2 weights after the second expert's.
        # TRN2 has two HWDGE queues (SP + Activation), each processing
        # trigger (~0.6us) + transfer serially. Startup-critical pieces
        # (expert 0 + its Me slice) go first on the sync queue - measured:
        # the scalar queue's first transfers complete late, so it only
        # carries bulk loads needed later (the other experts' activations,
        # the rest of Me, dec2 weights), halving the mid-kernel stream.
        nk0, sg0 = NKE[e0], segs[0]
        for g in range(0, nk0, 6):
            nc.sync.dma_start(
                out=w1sb[e0][:, g : g + 6, :], in_=w1_in[e0][:, g : g + 6, :]
            )
            nc.sync.dma_start(
                out=xts[0][:, g : g + 6, :],
                in_=bass.AP(tensor=xT, offset=xoffs[0] + g * sg0,
                            ap=[[nk0 * sg0, 128], [1, 6 * sg0]]),
            )
        nc.sync.dma_start(out=bcsb, in_=bc_in[:, :])
        nc.sync.dma_start(
            out=mesb[:, e0 * 4 : (e0 + 1) * 4, :],
            in_=me_in[:, e0 * 4 : (e0 + 1) * 4, :],
        )
        assert e0 == 0
        nc.sync.dma_start(
            out=mesb[:, 4 : NE * 4, :], in_=me_in[:, 4 : NE * 4, :]
        )
        for ei, e in enumerate(EORDER):
            if ei == 0 or segs[ei] == 0:
                continue
            nk, sg = NKE[e], segs[ei]
            nc.sync.dma_start(out=w1sb[e][:, :, :], in_=w1_in[e][:, :, :])
            nc.sync.dma_start(
                out=xts[ei][:, :, :],
                in_=bass.AP(tensor=xT, offset=xoffs[ei],
                            ap=[[nk * sg, 128], [1, nk * sg]]),
            )
            if ei == 1:
                nc.sync.dma_start(out=dw2sb, in_=dw2_in[:, :, :])

        def dec2_tile(s):
            # one 128-token row tile of the decoder output; psums in pairs of
            # column chunks (two banks) so one copy drains 1024 columns,
            # alternating vector/scalar
            ot = otP.tile([128, NCLS], BF16, name="ot")
            for n in range(NCH):
                nw = min(512, NCLS - n * 512)
                ps = dPs.tile([128, 512], F32, name="dps")
                for i in range(2):
                    nc.tensor.matmul(
                        ps[:, :nw],
                        sig[:, 2 * i : 2 * i + 2, s : s + 128],
                        dw2sb[:, 2 * i : 2 * i + 2, n * 512 : n * 512 + nw],
                        start=(i == 0),
                        stop=(i == 1),
                        perf_mode=DR,
                    )
                if n % 2 == 0:
                    nc.vector.tensor_copy(
                        out=ot[:, n * 512 : n * 512 + nw], in_=ps[:, :nw]
                    )
                else:
                    nc.scalar.activation(
                        ot[:, n * 512 : n * 512 + nw], ps[:, :nw],
                        mybir.ActivationFunctionType.Copy,
                    )
                if s == TP - 128 and n == 3:
                    # final row tile: ship the first half while the second
                    # half is still being copied, shortening the tail drain
                    nc.sync.dma_start(
                        out=out[s : s + 128, :2048], in_=ot[:, :2048]
                    )
            if s == TP - 128:
                nc.sync.dma_start(
                    out=out[s : s + 128, 2048:], in_=ot[:, 2048:]
                )
            else:
                nc.sync.dma_start(out=out[s : s + 128, :], in_=ot[:, :])

        def stage2(e, tpos, tcw, h):
            # Me matmuls + tanh for one 512-token block (emitted one block
            # behind stage 1 so the last relu of the block is long done by
            # the time the i=1 matmuls need h chunk 3 - no PE bubble)
            for m2 in range(4):
                zs = zPs.tile([128, 512], F32, name="zps")
                for i in range(2):
                    nc.tensor.matmul(
                        zs[:, :tcw],
                        mesb[:, e * 4 + 2 * i : e * 4 + 2 * i + 2,
                             m2 * 128 : (m2 + 1) * 128],
                        h[:, 2 * i : 2 * i + 2, :tcw],
                        start=(i == 0),
                        stop=(i == 1),
                        perf_mode=DR,
                    )
                nc.scalar.activation(
                    sig[:, m2, tpos : tpos + tcw], zs[:, :tcw],
                    mybir.ActivationFunctionType.Tanh,
                    bias=bcsb[:, 16 + e * 4 + m2 : 16 + e * 4 + m2 + 1],
                    scale=0.5 / WS,
                )

        pos = 0
        next_s = 0
        prev = None          # (e, tpos, tcw, h) of the block awaiting stage 2
        for ei, e in enumerate(EORDER):
            if segs[ei] == 0:
                continue
            nk = NKE[e]
            sg = segs[ei]
            xt = xts[ei]
            for t0 in range(0, sg, 512):
                tcw = min(512, sg - t0)
                h = hP.tile([128, 4, 512], FP8, name="h")
                for m in range(4):
                    ps = w1Ps.tile([128, 512], F32, name="w1ps")
                    for kj in range(0, nk, 2):
                        nc.tensor.matmul(
                            ps[:, :tcw],
                            w1sb[e][:, kj : kj + 2, m * 128 : (m + 1) * 128],
                            xt[:, kj : kj + 2, t0 : t0 + tcw],
                            start=(kj == 0),
                            stop=(kj == nk - 2),
                            perf_mode=DR,
                        )
                    nc.scalar.activation(
                        h[:, m, :tcw], ps[:, :tcw],
                        mybir.ActivationFunctionType.Relu,
                        bias=bcsb[:, e * 4 + m : e * 4 + m + 1], scale=1.0 / WS,
                    )
                if prev is not None:
                    stage2(*prev)
                    done = prev[1] + prev[2]
                    # emit dec2 for every fully-tanh'd 128-token row tile so
                    # copies and output DMA overlap the next block's matmuls
                    while next_s + 128 <= done:
                        dec2_tile(next_s)
                        next_s += 128
                prev = (e, pos + t0, tcw, h)
            pos += segs[ei]
        stage2(*prev)
        while next_s + 128 <= TP:
            dec2_tile(next_s)
            next_s += 128

    import bass_rust

    bass_rust.generate_event_semaphores(nc)
    return nc


_NC_CACHE = {}


def _get_nc(segs):
    key = tuple(segs)
    if key not in _NC_CACHE:
        _NC_CACHE[key] = _build(key)
    return _NC_CACHE[key]


def _route(inputs):
    """Host gate + routing plan. Returns (segs in EORDER, idx, flat)."""
    f32 = np.float32
    x = np.asarray(inputs["fusion_hs"], f32)                  # [L, B, D]
    flat = np.transpose(x, (1, 0, 2)).reshape(B, KD)          # [B, 6D]
    logits = flat.astype(np.float64) @ np.asarray(
        inputs["gate_W"], f32
    ).astype(np.float64) + np.asarray(inputs["gate_b"], f32).astype(np.float64)
    am = np.argmax(logits, axis=1)
    idx = [np.where(am == e)[0] for e in range(NE)]
    segs = [(len(idx[e]) + NCORES - 1) // NCORES for e in EORDER]
    # shave device capacity to exactly B/NCORES tokens per core: a ragged
    # dec2 subtile costs full matmul time for a couple of tokens, so the
    # few overflow tokens are computed on host instead
    excess = sum(segs) - B // NCORES
    if excess > 0:
        segs[int(np.argmax(segs))] -= excess
    return segs, idx, flat, am


def _prep_inputs(inputs, segs, idx, flat):
    f32 = np.float32
    ws = f32(WS)

    w1_raw = [
        np.asarray(inputs["e0_W1"], f32),
        np.asarray(inputs["e1_W1"], f32),
        np.asarray(inputs["e2_W1"], f32),
        np.array(inputs["e3_W1"], f32, copy=True),
    ]
    w1_raw[3][: 3 * D] *= f32(np.asarray(inputs["e3_a"]).reshape(-1)[0])
    w1_raw[3][3 * D :] *= f32(np.asarray(inputs["e3_b"]).reshape(-1)[0])

    dw1 = np.asarray(inputs["dec_W1"], f32)
    db1 = np.asarray(inputs["dec_b1"], f32)

    common = {}
    bc_cols = np.empty((128, NE * 8), f32)
    me_host = np.empty((128, NE * 4, 512), E4)
    for e in range(NE):
        b1e = np.asarray(inputs[f"e{e}_b1"], f32)
        bc_cols[:, e * 4 : (e + 1) * 4] = b1e.reshape(4, 128).T
        mee = np.asarray(inputs[f"e{e}_W2"], f32) @ dw1        # [512hid, 512h2]
        cee = np.asarray(inputs[f"e{e}_b2"], f32) @ dw1 + db1  # [512]
        bc_cols[:, 16 + e * 4 : 16 + (e + 1) * 4] = 0.5 * cee.reshape(4, 128).T
        me_host[:, e * 4 : (e + 1) * 4, :] = (
            (ws * mee).reshape(4, 128, 512).transpose(1, 0, 2).astype(E4)
        )
        common[f"w1_{e}"] = np.ascontiguousarray(
            (ws * w1_raw[e]).astype(E4).reshape(NKE[e], 128, 512).transpose(1, 0, 2)
        )
    common["bc"] = bc_cols
    common["me"] = me_host
    common["dw2"] = np.ascontiguousarray(
        (ws * 0.5 * np.asarray(inputs["dec_W2"], f32))
        .astype(E4)
        .reshape(4, 128, NCLS)
        .transpose(1, 0, 2)
    )

    flatT = np.ascontiguousarray(flat.T.astype(E4))            # [3072, B]
    xtot = 128 * sum(NKE[e] * sg for e, sg in zip(EORDER, segs))
    in_maps = []
    overflow = []
    for c in range(NCORES):
        xbuf = np.empty(xtot, E4)
        xoff = 0
        for ei, e in enumerate(EORDER):
            ids = idx[e][c::NCORES]
            take = min(len(ids), segs[ei])
            sel = ids[:take]
            if segs[ei] > take:
                pad = np.full(segs[ei] - take, ids[0] if take else 0, np.int64)
                sel = np.concatenate([sel, pad])
            overflow.extend(ids[take:].tolist())
            nk = NKE[e]
            sg = segs[ei]
            blk = flatT[KLO[e] * 128 : (KLO[e] + nk) * 128, sel]   # [nk*128, sg]
            xbuf[xoff : xoff + 128 * nk * sg] = (
                blk.reshape(nk, 128, sg).transpose(1, 0, 2).ravel()
            )
            xoff += 128 * nk * sg
        m = dict(common)
        m["xT"] = xbuf
        in_maps.append(m)
    return in_maps, overflow


def _host_fwd(inputs, flat, am, toks):
    # exact fp32 reference math for the few overflow tokens
    f32 = np.float32
    res = np.empty((len(toks), NCLS), f32)
    a = f32(np.asarray(inputs["e3_a"]).reshape(-1)[0])
    b = f32(np.asarray(inputs["e3_b"]).reshape(-1)[0])
    dW1 = np.asarray(inputs["dec_W1"], f32)
    db1 = np.asarray(inputs["dec_b1"], f32)
    dW2 = np.asarray(inputs["dec_W2"], f32)
    db2 = np.asarray(inputs["dec_b2"], f32)
    for j, t in enumerate(toks):
        e = int(am[t])
        if e == 0:
            xin = flat[t, : 3 * D]
        elif e == 1:
            xin = flat[t, 3 * D :]
        elif e == 2:
            xin = flat[t]
        else:
            xin = np.concatenate([flat[t, : 3 * D] * a, flat[t, 3 * D :] * b])
        h = np.maximum(
            xin @ np.asarray(inputs[f"e{e}_W1"], f32)
            + np.asarray(inputs[f"e{e}_b1"], f32), 0
        )
        sel = h @ np.asarray(inputs[f"e{e}_W2"], f32) + np.asarray(
            inputs[f"e{e}_b2"], f32
        )
        z = 1.0 / (1.0 + np.exp(-(sel @ dW1 + db1)))
        res[j] = z @ dW2 + db2
    return res


def _finish(inputs, segs, idx, results, host_rows, overflow):
    f32 = np.float32
    dw2_f = np.asarray(inputs["dec_W2"], f32)
    bias_out = 0.5 * dw2_f.sum(axis=0) + np.asarray(inputs["dec_b2"], f32)
    out_full = np.empty((B, NCLS), f32)
    for c in range(NCORES):
        res = np.asarray(results[c]["out"]).astype(f32)        # [TP, NCLS]
        p = 0
        for ei, e in enumerate(EORDER):
            ids = idx[e][c::NCORES]
            take = min(len(ids), segs[ei])
            out_full[ids[:take]] = res[p : p + take]
            p += segs[ei]
    out_full *= f32(1.0 / WS)
    out_full += bias_out
    if overflow:
        out_full[overflow] = host_rows   # host rows already include the bias
    return out_full


def _run(inputs, trace=False, tmpdir=None):
    segs, idx, flat, am = _route(inputs)
    nc = _get_nc(segs)
    in_maps, overflow = _prep_inputs(inputs, segs, idx, flat)
    res = run_bass_kernel_spmd(
        nc, in_maps, core_ids=list(range(NCORES)), trace=trace, tmpdir=tmpdir
    )
    host_rows = _host_fwd(inputs, flat, am, overflow) if overflow else None
    out = _finish(inputs, segs, idx, res.results, host_rows, overflow)
    return out, res


def kernel(**inputs):
    out, _ = _run(inputs)
    return out
